# revision 4
# baseline (speedup 1.0000x reference)
"""Trainium2 Bass kernel for nn_EnetGnn (GNN message passing).

Reference computation (per batch n, with X = rgb_in[n] viewed as (C=1024, HW=1024),
nodes = columns of X):
  S[i,j]   = x_i . x_j                       (node similarity, f32)
  nb(i)    = 16 smallest entries of S[i,:]   (k-NN, torch topk largest=False)
  M[m,:]   = relu(relu(X0_node_m @ w1 + b1) @ w2 + b2)   (MLP table; the
             reference gathers from the *globally flattened* node table, i.e.
             always batch 0's nodes)
  g_i      = mean_{m in nb(i)} M[m,:]
  A[i,j]   = g_i . g_j ; softmax over axis i (columns normalized)
  out      = X @ A_softmax + X

Implementation (8 cores, SPMD, one compiled program): core c handles batch
n = c//2 and channel-half h = c%2.  Each core computes the full
S/topk/MLP/G/A pipeline for its batch (duplicated within the pair) and the
final output for its 512-channel half.
  - top-16 per row via DVE max8 + match_replace (2 rounds), mask via not_equal
  - neighbor mean as a matmul with the 0/1 mask (P^T), M scaled by 1/16
  - softmax over the partition axis: exp on ACT, column sums via ones-vector
    matmul on PE, normalization applied after the output matmul
"""

import numpy as np
from contextlib import ExitStack

from concourse import mybir, bacc, tile
from concourse.bass import ts
from concourse.bass_utils import run_bass_kernel_spmd
from concourse.masks import make_identity

F32 = mybir.dt.float32
BF16 = mybir.dt.bfloat16
P = 128
HWDIM = 1024   # number of nodes per batch (H*W)
CDIM = 1024    # channels
FDIM = 256     # MLP hidden dim
NB = 4         # batch
NCORES = 8
CH = CDIM // 2  # channel half handled per core
MINVAL = -1.0e30

Copy = mybir.ActivationFunctionType.Copy
Relu = mybir.ActivationFunctionType.Relu
Exp = mybir.ActivationFunctionType.Exp


def _build_program(nc: bacc.Bacc):
    x = nc.dram_tensor("x", [CDIM, HWDIM], F32, kind="ExternalInput").ap()
    xh = nc.dram_tensor("xh", [CH, HWDIM], F32, kind="ExternalInput").ap()
    x0 = nc.dram_tensor("x0", [CDIM, HWDIM], F32, kind="ExternalInput").ap()
    w1 = nc.dram_tensor("w1", [CDIM, FDIM], F32, kind="ExternalInput").ap()
    w2 = nc.dram_tensor("w2", [FDIM, CDIM], F32, kind="ExternalInput").ap()
    b1 = nc.dram_tensor("b1", [2, P, 1], F32, kind="ExternalInput").ap()
    b2 = nc.dram_tensor("b2", [1, CDIM], F32, kind="ExternalInput").ap()
    out = nc.dram_tensor("out", [CH, HWDIM], F32, kind="ExternalOutput").ap()

    with tile.TileContext(nc) as tc, ExitStack() as ctx:
        persist = ctx.enter_context(tc.tile_pool(name="persist", bufs=1))

        # ---- constants ----
        id_f = persist.tile([P, P], F32, tag="id_f", name="id_f")
        make_identity(nc, id_f[:])
        id_b = persist.tile([P, P], BF16, tag="id_b", name="id_b")
        make_identity(nc, id_b[:])
        ones_row = persist.tile([1, P], F32, tag="ones_row", name="ones_row")
        nc.vector.memset(ones_row[:], 1.0)
        ones_col_b = persist.tile([P, 1], BF16, tag="ones_col_b", name="ones_col_b")
        nc.vector.memset(ones_col_b[:], 1.0)

        # ---- persistent sbuf buffers ----
        xh_sb = [persist.tile([P, HWDIM], F32, tag=f"xh{i}", name=f"xh{i}")
                 for i in range(4)]
        w1_sb = [persist.tile([P, FDIM], F32, tag=f"w1{i}", name=f"w1{i}")
                 for i in range(8)]
        w2b = [persist.tile([P, CDIM], BF16, tag=f"w2b{i}", name=f"w2b{i}")
               for i in range(2)]
        b1t = [persist.tile([P, 1], F32, tag=f"b1t{i}", name=f"b1t{i}")
               for i in range(2)]
        b2row = persist.tile([1, CDIM], F32, tag="b2row", name="b2row")
        h1t = [persist.tile([P, HWDIM], BF16, tag=f"h1t{i}", name=f"h1t{i}")
               for i in range(2)]
        m_sb = [persist.tile([P, CDIM], BF16, tag=f"m{i}", name=f"m{i}")
                for i in range(8)]
        pt_sb = [persist.tile([P, HWDIM], BF16, tag=f"pt{i}", name=f"pt{i}")
                 for i in range(8)]
        gt_sb = [persist.tile([P, HWDIM], BF16, tag=f"gt{i}", name=f"gt{i}")
                 for i in range(8)]
        e_sb = [persist.tile([P, HWDIM], BF16, tag=f"e{i}", name=f"e{i}")
                for i in range(8)]
        r_sb = [persist.tile([P, CH], BF16, tag=f"r{i}", name=f"r{i}")
                for i in range(8)]

        # ---- loads for persistent inputs ----
        for i in range(4):
            nc.sync.dma_start(xh_sb[i][:], xh[ts(i, P), :])
        for i in range(8):
            nc.sync.dma_start(w1_sb[i][:], w1[ts(i, P), :])
        for i in range(2):
            nc.sync.dma_start(b1t[i][:], b1[i])
        nc.sync.dma_start(b2row[:], b2[:, :])

        with ExitStack() as s1:
            # scoped sbuf: x resident; x0/w2 streamed; topk scratch; P masks
            sx = s1.enter_context(tc.tile_pool(name="sx", bufs=1))
            x_sb = [sx.tile([P, HWDIM], F32, tag=f"x{i}", name=f"x{i}")
                    for i in range(8)]
            pmask = [sx.tile([P, HWDIM], BF16, tag=f"pm{i}", name=f"pm{i}")
                     for i in range(8)]
            stream = s1.enter_context(tc.tile_pool(name="stream", bufs=2))
            topk_pool = s1.enter_context(tc.tile_pool(name="topk", bufs=2))

            for i in range(8):
                nc.sync.dma_start(x_sb[i][:], x[ts(i, P), :])
            # w2 -> bf16 via a single streamed staging slot
            for i in range(2):
                w2f = stream.tile([P, CDIM], F32, tag="w2f", name="w2f", bufs=1)
                nc.sync.dma_start(w2f[:], w2[ts(i, P), :])
                nc.scalar.activation(w2b[i][:], w2f[:], Copy)

            with ExitStack() as ps1:
                ps_s = ps1.enter_context(
                    tc.tile_pool(name="ps_s", bufs=2, space="PSUM"))
                ps_hm = ps1.enter_context(
                    tc.tile_pool(name="ps_hm", bufs=4, space="PSUM"))

                # ---- stage 1: S = X^T X (f32), top-16-smallest mask per row ----
                for t in range(8):
                    ps = ps_s.tile([P, HWDIM], F32, tag="S")
                    for cc in range(8):
                        lhsT = x_sb[cc][:, ts(t, P)]
                        for jh in range(2):
                            nc.tensor.matmul(
                                ps[:, ts(jh, 512)], lhsT, x_sb[cc][:, ts(jh, 512)],
                                start=(cc == 0), stop=(cc == 7),
                            )
                    sneg = topk_pool.tile([P, HWDIM], F32, tag="sneg", name="sneg")
                    nc.scalar.activation(sneg[:], ps[:], Copy, scale=-1.0)
                    m8a = topk_pool.tile([P, 8], F32, tag="m8a", name="m8a")
                    m8b = topk_pool.tile([P, 8], F32, tag="m8b", name="m8b")
                    szap = topk_pool.tile([P, HWDIM], F32, tag="szap", name="szap")
                    nc.vector.max(out=m8a[:], in_=sneg[:])
                    nc.vector.match_replace(
                        out=szap[:], in_to_replace=m8a[:], in_values=sneg[:],
                        imm_value=MINVAL,
                    )
                    nc.vector.max(out=m8b[:], in_=szap[:])
                    nc.vector.match_replace(
                        out=szap[:], in_to_replace=m8b[:], in_values=szap[:],
                        imm_value=MINVAL,
                    )
                    # 1.0 exactly at the 16 replaced positions
                    nc.vector.tensor_tensor(
                        out=pmask[t][:], in0=sneg[:], in1=szap[:],
                        op=mybir.AluOpType.not_equal,
                    )

                # ---- stage 2: MLP table M (m, c) from batch-0 nodes, /16 ----
                # H1T: all 4 psum groups accumulate while x0 tiles stream through
                hps = [ps_hm.tile([P, 512], F32, tag="HM", name=f"hps{k}")
                       for k in range(4)]
                for cc in range(8):
                    x0t = stream.tile([P, HWDIM], F32, tag="x0t", name="x0t")
                    nc.sync.dma_start(x0t[:], x0[ts(cc, P), :])
                    for ft in range(2):
                        for ih in range(2):
                            nc.tensor.matmul(
                                hps[ft * 2 + ih][:], w1_sb[cc][:, ts(ft, P)],
                                x0t[:, ts(ih, 512)],
                                start=(cc == 0), stop=(cc == 7),
                            )
                for ft in range(2):
                    for ih in range(2):
                        nc.scalar.activation(
                            h1t[ft][:, ts(ih, 512)], hps[ft * 2 + ih][:], Relu,
                            bias=b1t[ft][:],
                        )
                for mt in range(8):
                    for chh in range(2):
                        ps = ps_hm.tile([P, 512], F32, tag="HM", name="mps")
                        nc.tensor.matmul(ps[:], h1t[0][:, ts(mt, P)],
                                         w2b[0][:, ts(chh, 512)],
                                         start=True, stop=False)
                        nc.tensor.matmul(ps[:], h1t[1][:, ts(mt, P)],
                                         w2b[1][:, ts(chh, 512)],
                                         start=False, stop=False)
                        # + b2 broadcast along partitions via rank-1 matmul
                        nc.tensor.matmul(ps[:], ones_row[:],
                                         b2row[0:1, ts(chh, 512)],
                                         start=False, stop=True)
                        # relu(ps)/16 == relu(ps/16)
                        nc.scalar.activation(
                            m_sb[mt][:, ts(chh, 512)], ps[:], Relu,
                            scale=1.0 / 16.0,
                        )

            # ---- stage 3: P^T via PE transposes (bf16) ----
            with ExitStack() as ps2:
                ps_t = ps2.enter_context(
                    tc.tile_pool(name="ps_t", bufs=2, space="PSUM"))
                for mt in range(8):
                    ps = ps_t.tile([P, HWDIM], BF16, tag="PT")
                    for t in range(8):
                        nc.tensor.transpose(
                            ps[:, ts(t, P)], pmask[t][:, ts(mt, P)], id_b[:],
                        )
                    nc.scalar.activation(pt_sb[mt][:], ps[:], Copy)

        # ---- late-lived small tiles ----
        late = ctx.enter_context(tc.tile_pool(name="late", bufs=1))
        inv_row = late.tile([1, HWDIM], F32, tag="inv_row", name="inv_row")
        invbc = late.tile([P, HWDIM], F32, tag="invbc", name="invbc")

        # ---- stage 4: G^T (c, i) = sum_m M[m, c-slice] P^T[m, i] ----
        with ExitStack() as s3:
            ps_g = s3.enter_context(tc.tile_pool(name="ps_g", bufs=2, space="PSUM"))
            ps_r = s3.enter_context(tc.tile_pool(name="ps_r", bufs=2, space="PSUM"))
            for ct in range(8):
                for ih in range(2):
                    ps = ps_g.tile([P, 512], F32, tag="G")
                    for mt in range(8):
                        nc.tensor.matmul(
                            ps[:], m_sb[mt][:, ts(ct, P)], pt_sb[mt][:, ts(ih, 512)],
                            start=(mt == 0), stop=(mt == 7),
                        )
                    nc.scalar.activation(gt_sb[ct][:, ts(ih, 512)], ps[:], Copy)

            # ---- stage 5: R (m, c-half) = transpose of xh (bf16 out) ----
            for mt in range(8):
                ps = ps_r.tile([P, CH], F32, tag="R")
                for q in range(4):
                    nc.tensor.transpose(ps[:, ts(q, P)], xh_sb[q][:, ts(mt, P)],
                                        id_f[:])
                nc.scalar.activation(r_sb[mt][:], ps[:], Copy)

        # ---- stage 6: A (m, j) = G^T.T G^T, E = exp(A), column sums ----
        with ExitStack() as s4:
            ps_a = s4.enter_context(tc.tile_pool(name="ps_a", bufs=2, space="PSUM"))
            ps_cs = s4.enter_context(tc.tile_pool(name="ps_cs", bufs=1, space="PSUM"))
            cs = ps_cs.tile([1, HWDIM], F32, tag="CS")
            for mt in range(8):
                ps = ps_a.tile([P, HWDIM], F32, tag="A")
                for cc in range(8):
                    lhsT = gt_sb[cc][:, ts(mt, P)]
                    for jh in range(2):
                        nc.tensor.matmul(
                            ps[:, ts(jh, 512)], lhsT, gt_sb[cc][:, ts(jh, 512)],
                            start=(cc == 0), stop=(cc == 7),
                        )
                nc.scalar.activation(e_sb[mt][:], ps[:], Exp)
                for jh in range(2):
                    nc.tensor.matmul(
                        cs[0:1, ts(jh, 512)], ones_col_b[:], e_sb[mt][:, ts(jh, 512)],
                        start=(mt == 0), stop=(mt == 7),
                    )
            nc.vector.reciprocal(inv_row[:], cs[:])

        # ---- stage 7: broadcast inv along partitions; OUT = Id @ E ----
        with ExitStack() as s5:
            ps_o = s5.enter_context(tc.tile_pool(name="ps_o", bufs=2, space="PSUM"))
            ps_ib = s5.enter_context(tc.tile_pool(name="ps_ib", bufs=1, space="PSUM"))
            fin_pool = s5.enter_context(tc.tile_pool(name="fin", bufs=2))
            ib = ps_ib.tile([P, HWDIM], F32, tag="IB")
            for jh in range(2):
                nc.tensor.matmul(ib[:, ts(jh, 512)], ones_row[:],
                                 inv_row[0:1, ts(jh, 512)], start=True, stop=True)
            nc.vector.tensor_copy(out=invbc[:], in_=ib[:])

            for ct in range(4):
                ps = ps_o.tile([P, HWDIM], F32, tag="O")
                for mt in range(8):
                    lhsT = r_sb[mt][:, ts(ct, P)]
                    for jh in range(2):
                        nc.tensor.matmul(
                            ps[:, ts(jh, 512)], lhsT, e_sb[mt][:, ts(jh, 512)],
                            start=(mt == 0), stop=(mt == 7),
                        )
                tmp = fin_pool.tile([P, HWDIM], F32, tag="tmp", name="tmp")
                nc.vector.tensor_tensor(out=tmp[:], in0=ps[:], in1=invbc[:],
                                        op=mybir.AluOpType.mult)
                outt = fin_pool.tile([P, HWDIM], F32, tag="outt", name="outt")
                nc.vector.tensor_tensor(out=outt[:], in0=tmp[:], in1=xh_sb[ct][:],
                                        op=mybir.AluOpType.add)
                nc.sync.dma_start(out[ts(ct, P), :], outt[:])

    return nc


_NC = None


def _get_nc():
    global _NC
    if _NC is None:
        nc = bacc.Bacc("TRN2", target_bir_lowering=False, debug=False,
                       num_devices=NCORES)
        _build_program(nc)
        nc.compile()
        _NC = nc
    return _NC


def _in_maps(cat, rgb_in, w1, b1, w2, b2):
    del cat  # unused by the reference computation
    x4 = np.ascontiguousarray(rgb_in.reshape(NB, CDIM, HWDIM)).astype(np.float32)
    w1 = np.ascontiguousarray(w1, dtype=np.float32)
    w2 = np.ascontiguousarray(w2, dtype=np.float32)
    b1r = np.ascontiguousarray(b1.reshape(2, P, 1), dtype=np.float32)
    b2r = np.ascontiguousarray(b2.reshape(1, CDIM), dtype=np.float32)
    maps = []
    for core in range(NCORES):
        n, h = core // 2, core % 2
        maps.append({
            "x": x4[n],
            "xh": np.ascontiguousarray(x4[n, h * CH:(h + 1) * CH, :]),
            "x0": x4[0],
            "w1": w1,
            "w2": w2,
            "b1": b1r,
            "b2": b2r,
        })
    return maps


def _assemble(results, rgb_shape):
    N, C, H, W = rgb_shape
    out = np.empty((N, C, H * W), np.float32)
    for core, res in enumerate(results):
        n, h = core // 2, core % 2
        out[n, h * CH:(h + 1) * CH, :] = res["out"]
    return out.reshape(N, C, H, W)


def run_on_hw(cat, rgb_in, w1, b1, w2, b2, trace=False, **kw):
    nc = _get_nc()
    maps = _in_maps(cat, rgb_in, w1, b1, w2, b2)
    res = run_bass_kernel_spmd(nc, maps, core_ids=list(range(NCORES)),
                               trace=trace, **kw)
    out = _assemble(res.results, rgb_in.shape)
    return out, res


def kernel(cat, rgb_in, w1, b1, w2, b2, gnn_iterations=1, k=16):
    assert int(gnn_iterations) == 1 and int(k) == 16
    cat = np.asarray(cat)
    rgb_in = np.asarray(rgb_in, dtype=np.float32)
    out, _ = run_on_hw(cat, rgb_in, np.asarray(w1), np.asarray(b1),
                       np.asarray(w2), np.asarray(b2))
    return out


# revision 5
# speedup vs baseline: 1.7226x; 1.7226x over previous
"""Trainium2 Bass kernel for nn_EnetGnn (GNN message passing).

Reference computation (per batch n, with X = rgb_in[n] viewed as (C=1024, HW=1024),
nodes = columns of X):
  S[i,j]   = x_i . x_j                       (node similarity)
  nb(i)    = 16 smallest entries of S[i,:]   (k-NN, torch topk largest=False)
  M[m,:]   = relu(relu(X0_node_m @ w1 + b1) @ w2 + b2)   (MLP table; the
             reference gathers from the *globally flattened* node table, i.e.
             always batch 0's nodes)
  g_i      = mean_{m in nb(i)} M[m,:]
  A[i,j]   = g_i . g_j ; softmax over axis i (columns normalized)
  out      = X @ A_softmax + X

Implementation (8 cores, SPMD, one compiled program): core c handles batch
n = c//2 and channel-half h = c%2.  Each core computes the full
S/topk/MLP/G/A pipeline for its batch (duplicated within the pair) and the
final output for its 512-channel half.
  - fp32 matmul is 4x the cost of bf16 on trn2 (LOW_HIGH two-pass), so all
    matmuls run in bf16 with f32 psum accumulation.  Validated numerically:
    the top-16 sets see ~80/65536 boundary flips, each worth ~1e-3 absolute
    on the output (rel err 4e-4 overall).
  - top-16 per row via DVE max8 + match_replace (2 rounds), mask via not_equal
  - neighbor mean as a matmul with the 0/1 mask (P^T), M scaled by 1/16
  - softmax over the partition axis: exp on ACT, column sums via ones-vector
    matmul on PE, normalization applied after the output matmul; the
    1/colsum row is partition-broadcast on GpSimd.
"""

import numpy as np
from contextlib import ExitStack

from concourse import mybir, bacc, tile
from concourse.bass import ts
from concourse.bass_utils import run_bass_kernel_spmd
from concourse.masks import make_identity

F32 = mybir.dt.float32
BF16 = mybir.dt.bfloat16
P = 128
HWDIM = 1024   # number of nodes per batch (H*W)
CDIM = 1024    # channels
FDIM = 256     # MLP hidden dim
NB = 4         # batch
NCORES = 8
CH = CDIM // 2  # channel half handled per core
MINVAL = -1.0e30

Copy = mybir.ActivationFunctionType.Copy
Relu = mybir.ActivationFunctionType.Relu
Exp = mybir.ActivationFunctionType.Exp


def _build_program(nc: bacc.Bacc, use_b2: bool):
    x = nc.dram_tensor("x", [CDIM, HWDIM], F32, kind="ExternalInput").ap()
    xh = nc.dram_tensor("xh", [CH, HWDIM], F32, kind="ExternalInput").ap()
    x0 = nc.dram_tensor("x0", [CDIM, HWDIM], F32, kind="ExternalInput").ap()
    w1 = nc.dram_tensor("w1", [CDIM, FDIM], F32, kind="ExternalInput").ap()
    w2 = nc.dram_tensor("w2", [FDIM, CDIM], F32, kind="ExternalInput").ap()
    b1 = nc.dram_tensor("b1", [2, P, 1], F32, kind="ExternalInput").ap()
    b2 = nc.dram_tensor("b2", [1, CDIM], F32, kind="ExternalInput").ap()
    out = nc.dram_tensor("out", [CH, HWDIM], F32, kind="ExternalOutput").ap()

    with tile.TileContext(nc) as tc, ExitStack() as ctx:
        persist = ctx.enter_context(tc.tile_pool(name="persist", bufs=1))

        # ---- constants ----
        id_f = persist.tile([P, P], F32, tag="id_f", name="id_f")
        make_identity(nc, id_f[:])
        id_b = persist.tile([P, P], BF16, tag="id_b", name="id_b")
        make_identity(nc, id_b[:])
        ones_row = persist.tile([1, P], F32, tag="ones_row", name="ones_row")
        nc.vector.memset(ones_row[:], 1.0)
        ones_col_b = persist.tile([P, 1], BF16, tag="ones_col_b", name="ones_col_b")
        nc.vector.memset(ones_col_b[:], 1.0)

        # ---- persistent sbuf buffers ----
        xh_sb = [persist.tile([P, HWDIM], F32, tag=f"xh{i}", name=f"xh{i}")
                 for i in range(4)]
        w1b = [persist.tile([P, FDIM], BF16, tag=f"w1b{i}", name=f"w1b{i}")
               for i in range(8)]
        w2b = [persist.tile([P, CDIM], BF16, tag=f"w2b{i}", name=f"w2b{i}")
               for i in range(2)]
        b1t = [persist.tile([P, 1], F32, tag=f"b1t{i}", name=f"b1t{i}")
               for i in range(2)]
        b2row = persist.tile([1, CDIM], F32, tag="b2row", name="b2row")
        h1t = [persist.tile([P, HWDIM], BF16, tag=f"h1t{i}", name=f"h1t{i}")
               for i in range(2)]
        m_sb = [persist.tile([P, CDIM], BF16, tag=f"m{i}", name=f"m{i}")
                for i in range(8)]
        pt_sb = [persist.tile([P, HWDIM], BF16, tag=f"pt{i}", name=f"pt{i}")
                 for i in range(8)]
        r_sb = [persist.tile([P, CH], BF16, tag=f"r{i}", name=f"r{i}")
                for i in range(8)]
        invbc = persist.tile([P, HWDIM], F32, tag="invbc", name="invbc")
        inv_row = persist.tile([1, HWDIM], F32, tag="inv_row", name="inv_row")

        with ExitStack() as s1:
            # scoped sbuf: bf16 x resident; f32 x / x0 / w staged through pools
            sx = s1.enter_context(tc.tile_pool(name="sx", bufs=1))
            xb = [sx.tile([P, HWDIM], BF16, tag=f"xb{i}", name=f"xb{i}")
                  for i in range(8)]
            pmask = [sx.tile([P, HWDIM], BF16, tag=f"pm{i}", name=f"pm{i}")
                     for i in range(8)]
            stream = s1.enter_context(tc.tile_pool(name="stream", bufs=3))
            topk_pool = s1.enter_context(tc.tile_pool(name="topk", bufs=2))

            # x tiles first (S starts as soon as tile 0 lands + casts)
            for i in range(8):
                xf = stream.tile([P, HWDIM], F32, tag="xf", name="xf")
                nc.sync.dma_start(xf[:], x[ts(i, P), :])
                nc.scalar.activation(xb[i][:], xf[:], Copy)
            for i in range(4):
                nc.sync.dma_start(xh_sb[i][:], xh[ts(i, P), :])
            for i in range(8):
                wf = stream.tile([P, FDIM], F32, tag="wf", name="wf", bufs=2)
                nc.sync.dma_start(wf[:], w1[ts(i, P), :])
                nc.scalar.activation(w1b[i][:], wf[:], Copy)
            for i in range(2):
                w2f = stream.tile([P, CDIM], F32, tag="w2f", name="w2f", bufs=2)
                nc.sync.dma_start(w2f[:], w2[ts(i, P), :])
                nc.scalar.activation(w2b[i][:], w2f[:], Copy)
            for i in range(2):
                nc.sync.dma_start(b1t[i][:], b1[i])
            nc.sync.dma_start(b2row[:], b2[:, :])

            with ExitStack() as ps1:
                ps_s = ps1.enter_context(
                    tc.tile_pool(name="ps_s", bufs=2, space="PSUM"))
                ps_hm = ps1.enter_context(
                    tc.tile_pool(name="ps_hm", bufs=4, space="PSUM"))

                # ---- stage 1: S = X^T X (bf16), top-16-smallest mask per row ----
                for t in range(8):
                    ps = ps_s.tile([P, HWDIM], F32, tag="S")
                    for cc in range(8):
                        lhsT = xb[cc][:, ts(t, P)]
                        for jh in range(2):
                            nc.tensor.matmul(
                                ps[:, ts(jh, 512)], lhsT, xb[cc][:, ts(jh, 512)],
                                start=(cc == 0), stop=(cc == 7),
                            )
                    sneg = topk_pool.tile([P, HWDIM], F32, tag="sneg", name="sneg")
                    nc.scalar.activation(sneg[:], ps[:], Copy, scale=-1.0)
                    m8a = topk_pool.tile([P, 8], F32, tag="m8a", name="m8a")
                    m8b = topk_pool.tile([P, 8], F32, tag="m8b", name="m8b")
                    szap = topk_pool.tile([P, HWDIM], F32, tag="szap", name="szap")
                    nc.vector.max(out=m8a[:], in_=sneg[:])
                    nc.vector.match_replace(
                        out=szap[:], in_to_replace=m8a[:], in_values=sneg[:],
                        imm_value=MINVAL,
                    )
                    nc.vector.max(out=m8b[:], in_=szap[:])
                    nc.vector.match_replace(
                        out=szap[:], in_to_replace=m8b[:], in_values=szap[:],
                        imm_value=MINVAL,
                    )
                    # 1.0 exactly at the 16 replaced positions
                    nc.vector.tensor_tensor(
                        out=pmask[t][:], in0=sneg[:], in1=szap[:],
                        op=mybir.AluOpType.not_equal,
                    )

                # ---- stage 2: MLP table M (m, c) from batch-0 nodes, /16 ----
                # all 4 H1T psum groups accumulate while x0 tiles stream through
                hps = [ps_hm.tile([P, 512], F32, tag="HM", name=f"hps{k}")
                       for k in range(4)]
                for cc in range(8):
                    x0f = stream.tile([P, HWDIM], F32, tag="xf", name="x0f")
                    nc.sync.dma_start(x0f[:], x0[ts(cc, P), :])
                    x0t = stream.tile([P, HWDIM], BF16, tag="x0t", name="x0t")
                    nc.scalar.activation(x0t[:], x0f[:], Copy)
                    for ft in range(2):
                        for ih in range(2):
                            nc.tensor.matmul(
                                hps[ft * 2 + ih][:], w1b[cc][:, ts(ft, P)],
                                x0t[:, ts(ih, 512)],
                                start=(cc == 0), stop=(cc == 7),
                            )
                for ft in range(2):
                    for ih in range(2):
                        nc.scalar.activation(
                            h1t[ft][:, ts(ih, 512)], hps[ft * 2 + ih][:], Relu,
                            bias=b1t[ft][:],
                        )
                for mt in range(8):
                    for chh in range(2):
                        ps = ps_hm.tile([P, 512], F32, tag="HM", name="mps")
                        nc.tensor.matmul(ps[:], h1t[0][:, ts(mt, P)],
                                         w2b[0][:, ts(chh, 512)],
                                         start=True, stop=not use_b2)
                        nc.tensor.matmul(ps[:], h1t[1][:, ts(mt, P)],
                                         w2b[1][:, ts(chh, 512)],
                                         start=False, stop=False,
                                         skip_group_check=True)
                        if use_b2:
                            # + b2 broadcast along partitions via rank-1 matmul
                            nc.tensor.matmul(ps[:], ones_row[:],
                                             b2row[0:1, ts(chh, 512)],
                                             start=False, stop=True)
                        # relu(ps)/16 == relu(ps/16)
                        nc.scalar.activation(
                            m_sb[mt][:, ts(chh, 512)], ps[:], Relu,
                            scale=1.0 / 16.0,
                        )

            # ---- stage 3: P^T via PE transposes (bf16) ----
            with ExitStack() as ps2:
                ps_t = ps2.enter_context(
                    tc.tile_pool(name="ps_t", bufs=2, space="PSUM"))
                for mt in range(8):
                    ps = ps_t.tile([P, HWDIM], BF16, tag="PT")
                    for t in range(8):
                        nc.tensor.transpose(
                            ps[:, ts(t, P)], pmask[t][:, ts(mt, P)], id_b[:],
                        )
                    nc.scalar.activation(pt_sb[mt][:], ps[:], Copy)

        # ---- buffers that live only in the later stages ----
        late = ctx.enter_context(tc.tile_pool(name="late", bufs=1))
        gt_sb = [late.tile([P, HWDIM], BF16, tag=f"gt{i}", name=f"gt{i}")
                 for i in range(8)]
        e_sb = [late.tile([P, HWDIM], BF16, tag=f"e{i}", name=f"e{i}")
                for i in range(8)]

        # ---- stage 4: G^T (c, i) = sum_m M[m, c-slice] P^T[m, i] ----
        with ExitStack() as s3:
            ps_g = s3.enter_context(tc.tile_pool(name="ps_g", bufs=2, space="PSUM"))
            ps_r = s3.enter_context(tc.tile_pool(name="ps_r", bufs=2, space="PSUM"))
            for ct in range(8):
                for ih in range(2):
                    ps = ps_g.tile([P, 512], F32, tag="G")
                    for mt in range(8):
                        nc.tensor.matmul(
                            ps[:], m_sb[mt][:, ts(ct, P)], pt_sb[mt][:, ts(ih, 512)],
                            start=(mt == 0), stop=(mt == 7),
                        )
                    nc.scalar.activation(gt_sb[ct][:, ts(ih, 512)], ps[:], Copy)

            # ---- stage 5: R (m, c-half) = transpose of xh (bf16 out) ----
            for mt in range(8):
                ps = ps_r.tile([P, CH], F32, tag="R")
                for q in range(4):
                    nc.tensor.transpose(ps[:, ts(q, P)], xh_sb[q][:, ts(mt, P)],
                                        id_f[:])
                nc.scalar.activation(r_sb[mt][:], ps[:], Copy)

        # ---- stage 6: A (m, j) = G^T.T G^T, E = exp(A), column sums ----
        with ExitStack() as s4:
            ps_a = s4.enter_context(tc.tile_pool(name="ps_a", bufs=2, space="PSUM"))
            ps_cs = s4.enter_context(tc.tile_pool(name="ps_cs", bufs=1, space="PSUM"))
            cs = ps_cs.tile([1, HWDIM], F32, tag="CS")
            for mt in range(8):
                ps = ps_a.tile([P, HWDIM], F32, tag="A")
                for cc in range(8):
                    lhsT = gt_sb[cc][:, ts(mt, P)]
                    for jh in range(2):
                        nc.tensor.matmul(
                            ps[:, ts(jh, 512)], lhsT, gt_sb[cc][:, ts(jh, 512)],
                            start=(cc == 0), stop=(cc == 7),
                        )
                nc.scalar.activation(e_sb[mt][:], ps[:], Exp)
                for jh in range(2):
                    nc.tensor.matmul(
                        cs[0:1, ts(jh, 512)], ones_col_b[:], e_sb[mt][:, ts(jh, 512)],
                        start=(mt == 0), stop=(mt == 7),
                    )
            nc.vector.reciprocal(inv_row[:], cs[:])
            # broadcast 1/colsum to all partitions on the idle GpSimd engine
            nc.gpsimd.partition_broadcast(invbc[:], inv_row[0:1, :], channels=P)

        # ---- stage 7: OUT = Id @ E, scale by 1/colsum, add identity ----
        with ExitStack() as s5:
            ps_o = s5.enter_context(tc.tile_pool(name="ps_o", bufs=2, space="PSUM"))
            fin_pool = s5.enter_context(tc.tile_pool(name="fin", bufs=2))
            for ct in range(4):
                ps = ps_o.tile([P, HWDIM], F32, tag="O")
                for mt in range(8):
                    lhsT = r_sb[mt][:, ts(ct, P)]
                    for jh in range(2):
                        nc.tensor.matmul(
                            ps[:, ts(jh, 512)], lhsT, e_sb[mt][:, ts(jh, 512)],
                            start=(mt == 0), stop=(mt == 7),
                        )
                tmp = fin_pool.tile([P, HWDIM], F32, tag="tmp", name="tmp")
                nc.vector.tensor_tensor(out=tmp[:], in0=ps[:], in1=invbc[:],
                                        op=mybir.AluOpType.mult)
                outt = fin_pool.tile([P, HWDIM], F32, tag="outt", name="outt")
                nc.vector.tensor_tensor(out=outt[:], in0=tmp[:], in1=xh_sb[ct][:],
                                        op=mybir.AluOpType.add)
                nc.sync.dma_start(out[ts(ct, P), :], outt[:])

    return nc


_NC = {}


def _get_nc(use_b2=False):
    if use_b2 not in _NC:
        nc = bacc.Bacc("TRN2", target_bir_lowering=False, debug=False,
                       num_devices=NCORES)
        _build_program(nc, use_b2)
        nc.compile()
        _NC[use_b2] = nc
    return _NC[use_b2]


def _in_maps(cat, rgb_in, w1, b1, w2, b2):
    del cat  # unused by the reference computation
    x4 = np.ascontiguousarray(rgb_in.reshape(NB, CDIM, HWDIM)).astype(np.float32)
    w1 = np.ascontiguousarray(w1, dtype=np.float32)
    w2 = np.ascontiguousarray(w2, dtype=np.float32)
    b1r = np.ascontiguousarray(b1.reshape(2, P, 1), dtype=np.float32)
    b2r = np.ascontiguousarray(b2.reshape(1, CDIM), dtype=np.float32)
    maps = []
    for core in range(NCORES):
        n, h = core // 2, core % 2
        maps.append({
            "x": x4[n],
            "xh": np.ascontiguousarray(x4[n, h * CH:(h + 1) * CH, :]),
            "x0": x4[0],
            "w1": w1,
            "w2": w2,
            "b1": b1r,
            "b2": b2r,
        })
    return maps


def _assemble(results, rgb_shape):
    N, C, H, W = rgb_shape
    out = np.empty((N, C, H * W), np.float32)
    for core, res in enumerate(results):
        n, h = core // 2, core % 2
        out[n, h * CH:(h + 1) * CH, :] = res["out"]
    return out.reshape(N, C, H, W)


def run_on_hw(cat, rgb_in, w1, b1, w2, b2, trace=False, **kw):
    nc = _get_nc(use_b2=bool(np.any(np.asarray(b2))))
    maps = _in_maps(cat, rgb_in, w1, b1, w2, b2)
    res = run_bass_kernel_spmd(nc, maps, core_ids=list(range(NCORES)),
                               trace=trace, **kw)
    out = _assemble(res.results, rgb_in.shape)
    return out, res


def kernel(cat, rgb_in, w1, b1, w2, b2, gnn_iterations=1, k=16):
    assert int(gnn_iterations) == 1 and int(k) == 16
    cat = np.asarray(cat)
    rgb_in = np.asarray(rgb_in, dtype=np.float32)
    out, _ = run_on_hw(cat, rgb_in, np.asarray(w1), np.asarray(b1),
                       np.asarray(w2), np.asarray(b2))
    return out


# revision 11
# speedup vs baseline: 1.8046x; 1.0476x over previous
"""Trainium2 Bass kernel for nn_EnetGnn (GNN message passing).

Reference computation (per batch n, with X = rgb_in[n] viewed as (C=1024, HW=1024),
nodes = columns of X):
  S[i,j]   = x_i . x_j                       (node similarity)
  nb(i)    = 16 smallest entries of S[i,:]   (k-NN, torch topk largest=False)
  M[m,:]   = relu(relu(X0_node_m @ w1 + b1) @ w2 + b2)   (MLP table; the
             reference gathers from the *globally flattened* node table, i.e.
             always batch 0's nodes)
  g_i      = mean_{m in nb(i)} M[m,:]
  A[i,j]   = g_i . g_j ; softmax over axis i (columns normalized)
  out      = X @ A_softmax + X

Implementation (8 cores, SPMD, one compiled program): core c handles batch
n = c//2 and channel-half h = c%2.  Each core computes the full
S/topk/MLP/G/A pipeline for its batch (duplicated within the pair) and the
final output for its 512-channel half.
  - fp32 matmul is 4x the cost of bf16 on trn2 (LOW_HIGH two-pass), so all
    matmuls run in bf16 with f32 psum accumulation.  Validated numerically:
    the top-16 sets see ~80/65536 boundary flips, each worth ~1e-3 absolute
    on the output (rel err 4e-4 overall).
  - top-16 per row via DVE max8 + match_replace (2 rounds), mask via not_equal
  - neighbor mean as a matmul with the 0/1 mask (P^T), M scaled by 1/16
  - softmax over the partition axis: exp on ACT, column sums via ones-vector
    matmul on PE, normalization applied after the output matmul; the
    1/colsum row is partition-broadcast on GpSimd.
"""

import numpy as np
from contextlib import ExitStack

from concourse import mybir, bacc, tile
from concourse.bass import ts
from concourse.bass_utils import run_bass_kernel_spmd
from concourse.masks import make_identity

F32 = mybir.dt.float32
BF16 = mybir.dt.bfloat16
P = 128
HWDIM = 1024   # number of nodes per batch (H*W)
CDIM = 1024    # channels
FDIM = 256     # MLP hidden dim
NB = 4         # batch
NCORES = 8
CH = CDIM // 2  # channel half handled per core
MINVAL = -1.0e30

Copy = mybir.ActivationFunctionType.Copy
Relu = mybir.ActivationFunctionType.Relu
Exp = mybir.ActivationFunctionType.Exp


def _build_program(nc: bacc.Bacc, use_b2: bool):
    x = nc.dram_tensor("x", [CDIM, HWDIM], F32, kind="ExternalInput").ap()
    xh = nc.dram_tensor("xh", [CH, HWDIM], F32, kind="ExternalInput").ap()
    x0 = nc.dram_tensor("x0", [CDIM, HWDIM], F32, kind="ExternalInput").ap()
    w1 = nc.dram_tensor("w1", [CDIM, FDIM], F32, kind="ExternalInput").ap()
    w2 = nc.dram_tensor("w2", [FDIM, CDIM], F32, kind="ExternalInput").ap()
    b1 = nc.dram_tensor("b1", [2, P, 1], F32, kind="ExternalInput").ap()
    b2 = nc.dram_tensor("b2", [1, CDIM], F32, kind="ExternalInput").ap()
    out = nc.dram_tensor("out", [CH, HWDIM], F32, kind="ExternalOutput").ap()

    with tile.TileContext(nc) as tc, ExitStack() as ctx:
        persist = ctx.enter_context(tc.tile_pool(name="persist", bufs=1))

        # ---- constants ----
        id_f = persist.tile([P, P], F32, tag="id_f", name="id_f")
        make_identity(nc, id_f[:])
        id_b = persist.tile([P, P], BF16, tag="id_b", name="id_b")
        make_identity(nc, id_b[:])
        ones_row = persist.tile([1, P], F32, tag="ones_row", name="ones_row")
        nc.vector.memset(ones_row[:], 1.0)
        ones_col_b = persist.tile([P, 1], BF16, tag="ones_col_b", name="ones_col_b")
        nc.vector.memset(ones_col_b[:], 1.0)

        # ---- persistent sbuf buffers ----
        xh_sb = [persist.tile([P, HWDIM], F32, tag=f"xh{i}", name=f"xh{i}")
                 for i in range(4)]
        w1b = [persist.tile([P, FDIM], BF16, tag=f"w1b{i}", name=f"w1b{i}")
               for i in range(8)]
        w2b = [persist.tile([P, CDIM], BF16, tag=f"w2b{i}", name=f"w2b{i}")
               for i in range(2)]
        b1t = [persist.tile([P, 1], F32, tag=f"b1t{i}", name=f"b1t{i}")
               for i in range(2)]
        b2row = persist.tile([1, CDIM], F32, tag="b2row", name="b2row")
        h1t = [persist.tile([P, HWDIM], BF16, tag=f"h1t{i}", name=f"h1t{i}")
               for i in range(2)]
        m_sb = [persist.tile([P, CDIM], BF16, tag=f"m{i}", name=f"m{i}")
                for i in range(8)]
        pt_sb = [persist.tile([P, HWDIM], BF16, tag=f"pt{i}", name=f"pt{i}")
                 for i in range(8)]
        r_sb = [persist.tile([P, CH], BF16, tag=f"r{i}", name=f"r{i}")
                for i in range(8)]
        invbc = persist.tile([P, HWDIM], F32, tag="invbc", name="invbc")
        inv_row = persist.tile([1, HWDIM], F32, tag="inv_row", name="inv_row")

        with ExitStack() as s1:
            # scoped sbuf: bf16 x resident; f32 x / x0 / w staged through pools
            sx = s1.enter_context(tc.tile_pool(name="sx", bufs=1))
            xb = [sx.tile([P, HWDIM], BF16, tag=f"xb{i}", name=f"xb{i}")
                  for i in range(8)]
            pmask = [sx.tile([P, HWDIM], BF16, tag=f"pm{i}", name=f"pm{i}")
                     for i in range(8)]
            stream = s1.enter_context(tc.tile_pool(name="stream", bufs=3))
            topk_pool = s1.enter_context(tc.tile_pool(name="topk", bufs=2))

            # x tiles first (S starts as soon as tile 0 lands + casts);
            # alternate cast engine so the cast chain isn't ACT-serial
            for i in range(8):
                xf = stream.tile([P, HWDIM], F32, tag="xf", name="xf")
                nc.sync.dma_start(xf[:], x[ts(i, P), :])
                if i % 2 == 0:
                    nc.scalar.activation(xb[i][:], xf[:], Copy)
                else:
                    nc.vector.tensor_copy(out=xb[i][:], in_=xf[:])
            for i in range(4):
                nc.sync.dma_start(xh_sb[i][:], xh[ts(i, P), :])
            for i in range(8):
                wf = stream.tile([P, FDIM], F32, tag="wf", name="wf", bufs=2)
                nc.sync.dma_start(wf[:], w1[ts(i, P), :])
                nc.scalar.activation(w1b[i][:], wf[:], Copy)
            for i in range(2):
                w2f = stream.tile([P, CDIM], F32, tag="w2f", name="w2f", bufs=2)
                nc.sync.dma_start(w2f[:], w2[ts(i, P), :])
                nc.scalar.activation(w2b[i][:], w2f[:], Copy)
            for i in range(2):
                nc.sync.dma_start(b1t[i][:], b1[i])
            nc.sync.dma_start(b2row[:], b2[:, :])

            with ExitStack() as ps1:
                ps_s = ps1.enter_context(
                    tc.tile_pool(name="ps_s", bufs=2, space="PSUM"))
                ps_hm = ps1.enter_context(
                    tc.tile_pool(name="ps_hm", bufs=4, space="PSUM"))

                # ---- stage 1: S = X^T X (bf16), top-16-smallest mask per row ----
                for t in range(8):
                    ps = ps_s.tile([P, HWDIM], F32, tag="S")
                    for cc in range(8):
                        lhsT = xb[cc][:, ts(t, P)]
                        for jh in range(2):
                            nc.tensor.matmul(
                                ps[:, ts(jh, 512)], lhsT, xb[cc][:, ts(jh, 512)],
                                start=(cc == 0), stop=(cc == 7),
                            )
                    sneg = topk_pool.tile([P, HWDIM], F32, tag="sneg", name="sneg")
                    nc.scalar.activation(sneg[:], ps[:], Copy, scale=-1.0)
                    m8a = topk_pool.tile([P, 8], F32, tag="m8a", name="m8a")
                    m8b = topk_pool.tile([P, 8], F32, tag="m8b", name="m8b")
                    szap = topk_pool.tile([P, HWDIM], F32, tag="szap", name="szap")
                    nc.vector.max(out=m8a[:], in_=sneg[:])
                    nc.vector.match_replace(
                        out=szap[:], in_to_replace=m8a[:], in_values=sneg[:],
                        imm_value=MINVAL,
                    )
                    nc.vector.max(out=m8b[:], in_=szap[:])
                    nc.vector.match_replace(
                        out=szap[:], in_to_replace=m8b[:], in_values=szap[:],
                        imm_value=MINVAL,
                    )
                    # 1.0 exactly at the 16 replaced positions
                    nc.vector.tensor_tensor(
                        out=pmask[t][:], in0=sneg[:], in1=szap[:],
                        op=mybir.AluOpType.not_equal,
                    )

                # ---- stage 2: MLP table M (m, c) from batch-0 nodes, /16 ----
                # all 4 H1T psum groups accumulate while x0 tiles stream through
                hps = [ps_hm.tile([P, 512], F32, tag="HM", name=f"hps{k}")
                       for k in range(4)]
                for cc in range(8):
                    x0f = stream.tile([P, HWDIM], F32, tag="xf", name="x0f")
                    nc.sync.dma_start(x0f[:], x0[ts(cc, P), :])
                    x0t = stream.tile([P, HWDIM], BF16, tag="x0t", name="x0t")
                    nc.scalar.activation(x0t[:], x0f[:], Copy)
                    for ft in range(2):
                        for ih in range(2):
                            nc.tensor.matmul(
                                hps[ft * 2 + ih][:], w1b[cc][:, ts(ft, P)],
                                x0t[:, ts(ih, 512)],
                                start=(cc == 0), stop=(cc == 7),
                            )
                for ft in range(2):
                    for ih in range(2):
                        nc.scalar.activation(
                            h1t[ft][:, ts(ih, 512)], hps[ft * 2 + ih][:], Relu,
                            bias=b1t[ft][:],
                        )
                for mt in range(8):
                    for chh in range(2):
                        ps = ps_hm.tile([P, 512], F32, tag="HM", name="mps")
                        nc.tensor.matmul(ps[:], h1t[0][:, ts(mt, P)],
                                         w2b[0][:, ts(chh, 512)],
                                         start=True, stop=not use_b2)
                        nc.tensor.matmul(ps[:], h1t[1][:, ts(mt, P)],
                                         w2b[1][:, ts(chh, 512)],
                                         start=False, stop=False,
                                         skip_group_check=True)
                        if use_b2:
                            # + b2 broadcast along partitions via rank-1 matmul
                            nc.tensor.matmul(ps[:], ones_row[:],
                                             b2row[0:1, ts(chh, 512)],
                                             start=False, stop=True)
                        # relu(ps)/16 == relu(ps/16)
                        nc.scalar.activation(
                            m_sb[mt][:, ts(chh, 512)], ps[:], Relu,
                            scale=1.0 / 16.0,
                        )

            # ---- stage 3: P^T via PE transposes (bf16), by i-half so the
            # G^T half-0 matmuls can start while topk of tiles 4..7 still runs
            with ExitStack() as ps2:
                ps_t = ps2.enter_context(
                    tc.tile_pool(name="ps_t", bufs=4, space="PSUM"))
                for ih in range(2):
                    for mt in range(8):
                        ps = ps_t.tile([P, 512], BF16, tag="PT")
                        for q in range(4):
                            nc.tensor.transpose(
                                ps[:, ts(q, P)],
                                pmask[ih * 4 + q][:, ts(mt, P)], id_b[:],
                            )
                        nc.scalar.activation(pt_sb[mt][:, ts(ih, 512)], ps[:], Copy)

        # ---- buffers that live only in the later stages ----
        late = ctx.enter_context(tc.tile_pool(name="late", bufs=1))
        gt_sb = [late.tile([P, HWDIM], BF16, tag=f"gt{i}", name=f"gt{i}")
                 for i in range(8)]
        e_sb = [late.tile([P, HWDIM], BF16, tag=f"e{i}", name=f"e{i}")
                for i in range(8)]

        # ---- stage 4: G^T (c, i) = sum_m M[m, c-slice] P^T[m, i], by i-half ----
        with ExitStack() as s3:
            ps_g = s3.enter_context(tc.tile_pool(name="ps_g", bufs=4, space="PSUM"))
            ps_r = s3.enter_context(tc.tile_pool(name="ps_r", bufs=2, space="PSUM"))
            for ih in range(2):
                for ct in range(8):
                    ps = ps_g.tile([P, 512], F32, tag="G")
                    for mt in range(8):
                        nc.tensor.matmul(
                            ps[:], m_sb[mt][:, ts(ct, P)], pt_sb[mt][:, ts(ih, 512)],
                            start=(mt == 0), stop=(mt == 7),
                        )
                    nc.scalar.activation(gt_sb[ct][:, ts(ih, 512)], ps[:], Copy)

            # ---- stage 5: R (m, c-half) = transpose of xh (bf16 out) ----
            for mt in range(8):
                ps = ps_r.tile([P, CH], F32, tag="R")
                for q in range(4):
                    nc.tensor.transpose(ps[:, ts(q, P)], xh_sb[q][:, ts(mt, P)],
                                        id_f[:])
                nc.vector.tensor_copy(out=r_sb[mt][:], in_=ps[:])

        # ---- stage 6: A (m, j) = G^T.T G^T, E = exp(A), column sums ----
        # quadrant order: (mt<4, jh=0) needs only G^T half 0, so it starts
        # while G^T half 1 is still accumulating
        with ExitStack() as s4:
            ps_a = s4.enter_context(tc.tile_pool(name="ps_a", bufs=4, space="PSUM"))
            ps_cs = s4.enter_context(tc.tile_pool(name="ps_cs", bufs=1, space="PSUM"))
            cs = ps_cs.tile([1, HWDIM], F32, tag="CS")
            quads = ([(mt, 0) for mt in range(4)]
                     + [(mt, 1) for mt in range(4)]
                     + [(mt, jh) for mt in range(4, 8) for jh in range(2)])
            seen = [0, 0]
            for mt, jh in quads:
                ps = ps_a.tile([P, 512], F32, tag="A")
                for cc in range(8):
                    nc.tensor.matmul(
                        ps[:], gt_sb[cc][:, ts(mt, P)], gt_sb[cc][:, ts(jh, 512)],
                        start=(cc == 0), stop=(cc == 7),
                    )
                nc.scalar.activation(e_sb[mt][:, ts(jh, 512)], ps[:], Exp)
                nc.tensor.matmul(
                    cs[0:1, ts(jh, 512)], ones_col_b[:], e_sb[mt][:, ts(jh, 512)],
                    start=(seen[jh] == 0), stop=(seen[jh] == 7),
                )
                seen[jh] += 1
            nc.vector.reciprocal(inv_row[:], cs[:])
            # broadcast 1/colsum to all partitions on the idle GpSimd engine
            nc.gpsimd.partition_broadcast(invbc[:], inv_row[0:1, :], channels=P)

        # ---- stage 7: OUT = Id @ E, scale by 1/colsum, add identity ----
        with ExitStack() as s5:
            ps_o = s5.enter_context(tc.tile_pool(name="ps_o", bufs=4, space="PSUM"))
            fin_pool = s5.enter_context(tc.tile_pool(name="fin", bufs=4))
            for ct in range(4):
                for jh in range(2):
                    ps = ps_o.tile([P, 512], F32, tag="O")
                    for mt in range(8):
                        nc.tensor.matmul(
                            ps[:], r_sb[mt][:, ts(ct, P)], e_sb[mt][:, ts(jh, 512)],
                            start=(mt == 0), stop=(mt == 7),
                        )
                    tmp = fin_pool.tile([P, 512], F32, tag="tmp", name="tmp")
                    nc.vector.tensor_tensor(
                        out=tmp[:], in0=ps[:], in1=invbc[:, ts(jh, 512)],
                        op=mybir.AluOpType.mult)
                    outt = fin_pool.tile([P, 512], F32, tag="outt", name="outt")
                    nc.vector.tensor_tensor(
                        out=outt[:], in0=tmp[:], in1=xh_sb[ct][:, ts(jh, 512)],
                        op=mybir.AluOpType.add)
                    nc.sync.dma_start(out[ts(ct, P), ts(jh, 512)], outt[:])

    return nc


_NC = {}


def _get_nc(use_b2=False):
    if use_b2 not in _NC:
        nc = bacc.Bacc("TRN2", target_bir_lowering=False, debug=False,
                       num_devices=NCORES)
        _build_program(nc, use_b2)
        nc.compile()
        _NC[use_b2] = nc
    return _NC[use_b2]


def _in_maps(cat, rgb_in, w1, b1, w2, b2):
    del cat  # unused by the reference computation
    x4 = np.ascontiguousarray(rgb_in.reshape(NB, CDIM, HWDIM)).astype(np.float32)
    w1 = np.ascontiguousarray(w1, dtype=np.float32)
    w2 = np.ascontiguousarray(w2, dtype=np.float32)
    b1r = np.ascontiguousarray(b1.reshape(2, P, 1), dtype=np.float32)
    b2r = np.ascontiguousarray(b2.reshape(1, CDIM), dtype=np.float32)
    maps = []
    for core in range(NCORES):
        n, h = core // 2, core % 2
        maps.append({
            "x": x4[n],
            "xh": np.ascontiguousarray(x4[n, h * CH:(h + 1) * CH, :]),
            "x0": x4[0],
            "w1": w1,
            "w2": w2,
            "b1": b1r,
            "b2": b2r,
        })
    return maps


def _assemble(results, rgb_shape):
    N, C, H, W = rgb_shape
    out = np.empty((N, C, H * W), np.float32)
    for core, res in enumerate(results):
        n, h = core // 2, core % 2
        out[n, h * CH:(h + 1) * CH, :] = res["out"]
    return out.reshape(N, C, H, W)


def run_on_hw(cat, rgb_in, w1, b1, w2, b2, trace=False, **kw):
    nc = _get_nc(use_b2=bool(np.any(np.asarray(b2))))
    maps = _in_maps(cat, rgb_in, w1, b1, w2, b2)
    res = run_bass_kernel_spmd(nc, maps, core_ids=list(range(NCORES)),
                               trace=trace, **kw)
    out = _assemble(res.results, rgb_in.shape)
    return out, res


def kernel(cat, rgb_in, w1, b1, w2, b2, gnn_iterations=1, k=16):
    assert int(gnn_iterations) == 1 and int(k) == 16
    cat = np.asarray(cat)
    rgb_in = np.asarray(rgb_in, dtype=np.float32)
    out, _ = run_on_hw(cat, rgb_in, np.asarray(w1), np.asarray(b1),
                       np.asarray(w2), np.asarray(b2))
    return out
